# revision 1
# baseline (speedup 1.0000x reference)
"""DistMult scoring kernel for Trainium2 (8 NeuronCores, Bass/Tile).

reference computation:
    rel = rel_embeds[rel_ids]                      # [B, D] gather
    scores = sum(head * rel * tail, axis=-1)       # [B]
    pos = min(scores[:n_pos], upper_bound)
    neg = max(scores[n_pos:], lower_bound)
    out = sigmoid(concat(pos, neg))

Sharding: data-parallel over B. Core c owns rows [c*65536, (c+1)*65536).
Within a core, local row r maps to (partition p, column t) with r = p*512 + t,
which makes every stream DMA contiguous per partition and lets the final
[128, 512] score tile be stored with a single contiguous DMA.

The pos/neg split falls on a core boundary (131072 = 2 * 65536), handled
data-parallel by feeding cores +/-inf padded bounds:
    out = sigmoid(max(min(scores, ub), lb))
with ub=+inf for neg cores and lb=-inf for pos cores.
"""

import sys

for _p in ("/opt/trn_rl_repo",):
    if _p not in sys.path:
        sys.path.insert(0, _p)

import numpy as np

import concourse.bacc as bacc
import concourse.bass as bass
import concourse.mybir as mybir
import concourse.tile as tile
from concourse.bass_utils import run_bass_kernel_spmd

N_POS = 131072
N_NEG = 393216
B = N_POS + N_NEG  # 524288
D = 256
NUM_REL = 500
NCORES = 8
ROWS = B // NCORES  # 65536 rows per core
P = 128
T = ROWS // P  # 512 tiles of 128 rows; local row = p*T + t
GROUP = 8  # tiles per loop iteration
NG = T // GROUP  # 64 iterations

# stream dtype for head/tail/rel table ("f32" or "bf16")
STREAM_DT = "bf16"


def build_program(stream_dt: str = STREAM_DT):
    sdt = mybir.dt.float32 if stream_dt == "f32" else mybir.dt.bfloat16
    f32 = mybir.dt.float32
    i32 = mybir.dt.int32
    mult = mybir.AluOpType.mult
    add = mybir.AluOpType.add

    nc = bacc.Bacc(
        "TRN2", target_bir_lowering=False, debug=False, num_devices=NCORES
    )
    h = nc.declare_dram_parameter("h", [ROWS, D], sdt, isOutput=False)
    t_ = nc.declare_dram_parameter("t", [ROWS, D], sdt, isOutput=False)
    # pair ids: ids[p, 2u]*NUM_REL + ids[p, 2u+1], [ROWS//2] int32
    ids = nc.declare_dram_parameter("ids", [ROWS // 2], i32, isOutput=False)
    ub = nc.declare_dram_parameter("ub", [ROWS], f32, isOutput=False)
    lb = nc.declare_dram_parameter("lb", [ROWS], f32, isOutput=False)
    # pair table: row i*NUM_REL+j = concat(table[i], table[j])
    table = nc.declare_dram_parameter(
        "table", [NUM_REL * NUM_REL, 2 * D], sdt, isOutput=False
    )
    out = nc.declare_dram_parameter("out", [ROWS], f32, isOutput=True)

    h_v = h[:].rearrange("(p t) d -> p t d", p=P)
    t_v = t_[:].rearrange("(p t) d -> p t d", p=P)
    ids_v = ids[:].rearrange("(p t) -> p t", p=P)
    ub_v = ub[:].rearrange("(p t) -> p t", p=P)
    lb_v = lb[:].rearrange("(p t) -> p t", p=P)
    out_v = out[:].rearrange("(p t) -> p t", p=P)

    with tile.TileContext(nc) as tc:
        with (
            tc.tile_pool(name="io", bufs=1) as io_pool,
            tc.tile_pool(name="stream", bufs=4) as spool,
            tc.tile_pool(name="rpool", bufs=8) as rpool,
            tc.tile_pool(name="scratch", bufs=2) as qpool,
        ):
            ids_all = io_pool.tile([P, T // 2], i32)
            nc.sync.dma_start(out=ids_all[:], in_=ids_v)
            scores = io_pool.tile([P, T], f32)

            for g in range(NG):
                t0 = g * GROUP
                htile = spool.tile([P, GROUP * D], sdt, tag="h")
                ttile = spool.tile([P, GROUP * D], sdt, tag="t")
                rtile = rpool.tile([P, GROUP * D], sdt, tag="r")
                nc.sync.dma_start(
                    out=htile[:].rearrange("p (g d) -> p g d", g=GROUP),
                    in_=h_v[:, t0 : t0 + GROUP, :],
                )
                nc.sync.dma_start(
                    out=ttile[:].rearrange("p (g d) -> p g d", g=GROUP),
                    in_=t_v[:, t0 : t0 + GROUP, :],
                )
                u0 = t0 // 2
                for u in range(GROUP // 2):
                    nc.gpsimd.indirect_dma_start(
                        out=rtile[:, u * 2 * D : (u + 1) * 2 * D],
                        out_offset=None,
                        in_=table[:],
                        in_offset=bass.IndirectOffsetOnAxis(
                            ap=ids_all[:, u0 + u : u0 + u + 1], axis=0
                        ),
                    )
                q = qpool.tile([P, GROUP * D], sdt, tag="q")
                s = qpool.tile([P, GROUP * D], sdt, tag="s")
                nc.vector.tensor_tensor(
                    out=q[:], in0=htile[:], in1=ttile[:], op=mult
                )
                for gi in range(GROUP):
                    seg = slice(gi * D, (gi + 1) * D)
                    nc.vector.scalar_tensor_tensor(
                        out=s[:, seg],
                        in0=q[:, seg],
                        scalar=1.0,
                        in1=rtile[:, seg],
                        op0=mult,
                        op1=mult,
                        accum_out=scores[:, t0 + gi : t0 + gi + 1],
                    )

            # tail: clamp + sigmoid + store
            ubt = io_pool.tile([P, T], f32)
            lbt = io_pool.tile([P, T], f32)
            nc.sync.dma_start(out=ubt[:], in_=ub_v)
            nc.sync.dma_start(out=lbt[:], in_=lb_v)
            clip1 = io_pool.tile([P, T], f32)
            clip2 = io_pool.tile([P, T], f32)
            nc.vector.tensor_tensor(
                out=clip1[:], in0=scores[:], in1=ubt[:], op=mybir.AluOpType.min
            )
            nc.vector.tensor_tensor(
                out=clip2[:], in0=clip1[:], in1=lbt[:], op=mybir.AluOpType.max
            )
            sig = io_pool.tile([P, T], f32)
            nc.scalar.activation(
                out=sig[:], in_=clip2[:], func=mybir.ActivationFunctionType.Sigmoid
            )
            nc.sync.dma_start(out=out_v, in_=sig[:])

    nc.compile()
    return nc


def make_in_maps(inputs: dict, stream_dt: str = STREAM_DT):
    np_sdt = np.float32 if stream_dt == "f32" else None
    import ml_dtypes

    if np_sdt is None:
        np_sdt = ml_dtypes.bfloat16

    head = np.asarray(inputs["head_embeds"], dtype=np.float32)
    tail = np.asarray(inputs["tail_embeds"], dtype=np.float32)
    rel_ids = np.asarray(inputs["rel_ids"]).astype(np.int32)
    lower = np.asarray(inputs["lower_bound"], dtype=np.float32)
    upper = np.asarray(inputs["upper_bound"], dtype=np.float32)
    table1 = np.asarray(inputs["rel_embeds"], dtype=np.float32).astype(np_sdt)

    head = head.astype(np_sdt)
    tail = tail.astype(np_sdt)

    # pair table: row i*NUM_REL+j = [table[i] | table[j]]
    table = np.empty((NUM_REL * NUM_REL, 2 * D), dtype=np_sdt)
    table[:, :D] = np.repeat(table1, NUM_REL, axis=0)
    table[:, D:] = np.tile(table1, (NUM_REL, 1))

    pos_inf = np.full(ROWS, np.inf, dtype=np.float32)
    neg_inf = np.full(ROWS, -np.inf, dtype=np.float32)

    in_maps = []
    for c in range(NCORES):
        lo = c * ROWS
        hi = lo + ROWS
        if hi <= N_POS:
            ub_c = upper[lo:hi]
            lb_c = neg_inf
        else:
            assert lo >= N_POS
            ub_c = pos_inf
            lb_c = lower[lo - N_POS : hi - N_POS]
        # pair ids in (p, u) layout: local row r = p*T + t; pairs along t
        ids_c = rel_ids[lo:hi].reshape(P, T // 2, 2).astype(np.int64)
        pair_ids = (ids_c[:, :, 0] * NUM_REL + ids_c[:, :, 1]).astype(np.int32)
        in_maps.append(
            {
                "h": np.ascontiguousarray(head[lo:hi]),
                "t": np.ascontiguousarray(tail[lo:hi]),
                "ids": np.ascontiguousarray(pair_ids.reshape(-1)),
                "ub": np.ascontiguousarray(ub_c),
                "lb": np.ascontiguousarray(lb_c),
                "table": table,
            }
        )
    return in_maps


def kernel(**inputs) -> np.ndarray:
    nc = build_program(STREAM_DT)
    in_maps = make_in_maps(inputs, STREAM_DT)
    res = run_bass_kernel_spmd(nc, in_maps, list(range(NCORES)))
    return np.concatenate([res.results[c]["out"] for c in range(NCORES)])



# revision 2
# speedup vs baseline: 1.2626x; 1.2626x over previous
"""DistMult scoring kernel for Trainium2 (8 NeuronCores, Bass/Tile).

reference computation:
    rel = rel_embeds[rel_ids]                      # [B, D] gather
    scores = sum(head * rel * tail, axis=-1)       # [B]
    pos = min(scores[:n_pos], upper_bound)
    neg = max(scores[n_pos:], lower_bound)
    out = sigmoid(concat(pos, neg))

Strategy (sorted-chunk + selector-matmul; no device-side gather):
  * Host sorts rows by rel_id and packs them into chunks of CH=16 rows
    that all share one relation. Chunks are padded to a static layout:
    8 chunks per 128-row tile (chunk k -> partitions [16k, 16k+16)),
    T_PC=528 tiles per core. Pad slots carry zero h/t and +/-inf bounds,
    and are dropped on the host after the run.
  * Per tile the 8 chunk rel vectors [8, 256] are expanded to a full
    [128, 256] per-slot rel operand with a single TensorE matmul against
    a static one-hot selector lhsT [8, 128] (exact: one term per output).
    ScalarE casts the PSUM result to bf16 in SBUF.
  * DVE computes q = h*t and s = q*rel_bcast as big bf16 ops (2x mode)
    and reduces s over D with tensor_reduce into f32 scores.
  * Clamp with per-slot padded bounds (ub=+inf for neg rows, lb=-inf for
    pos rows), sigmoid, store. Host unpermutes.

Per-core slot r = p*T_PC + t (partition p, tile t) so every stream DMA
is contiguous per partition (16 tiles -> 8KB lines, 1MB per dma_start).
"""

import sys

for _p in ("/opt/trn_rl_repo",):
    if _p not in sys.path:
        sys.path.insert(0, _p)

import numpy as np

import concourse.bacc as bacc
import concourse.bass as bass
import concourse.mybir as mybir
import concourse.tile as tile
from concourse.bass_utils import run_bass_kernel_spmd

N_POS = 131072
N_NEG = 393216
B = N_POS + N_NEG  # 524288
D = 256
NUM_REL = 500
NCORES = 8
P = 128

CH = 16  # rows per chunk (uniform rel id within a chunk)
CPT = 8  # chunks per 128-row tile; chunk k -> partitions [16k, 16k+16)
T_PC = 528  # tiles per core
R = P * T_PC  # 67584 slots per core
GROUP = 16  # tiles per loop iteration
NG = T_PC // GROUP  # 33
CHUNKS_PER_CORE = T_PC * CPT  # 4224
TOTAL_CHUNKS = NCORES * CHUNKS_PER_CORE  # 33792 >= 500 + B/16 worst case


def build_program():
    bf = mybir.dt.bfloat16
    f32 = mybir.dt.float32
    mult = mybir.AluOpType.mult

    nc = bacc.Bacc(
        "TRN2", target_bir_lowering=False, debug=False, num_devices=NCORES
    )
    h = nc.declare_dram_parameter("h", [R, D], bf, isOutput=False)
    t_ = nc.declare_dram_parameter("t", [R, D], bf, isOutput=False)
    # rel[g, k, i*D:(i+1)*D] = rel vector of chunk k of tile g*GROUP+i
    rel = nc.declare_dram_parameter("rel", [NG, CPT, GROUP * D], bf, isOutput=False)
    # one-hot selector: sel[k, m] = 1 iff m//CH == k
    sel = nc.declare_dram_parameter("sel", [CPT, P], bf, isOutput=False)
    ub = nc.declare_dram_parameter("ub", [R], f32, isOutput=False)
    lb = nc.declare_dram_parameter("lb", [R], f32, isOutput=False)
    out = nc.declare_dram_parameter("out", [R], f32, isOutput=True)

    h_v = h[:].rearrange("(p t) d -> p t d", p=P)
    t_v = t_[:].rearrange("(p t) d -> p t d", p=P)
    rel_v = rel[:]
    ub_v = ub[:].rearrange("(p t) -> p t", p=P)
    lb_v = lb[:].rearrange("(p t) -> p t", p=P)
    out_v = out[:].rearrange("(p t) -> p t", p=P)

    with tile.TileContext(nc) as tc:
        with (
            tc.tile_pool(name="io", bufs=1) as io,
            tc.tile_pool(name="stream", bufs=3) as spool,
            tc.tile_pool(name="relp", bufs=2) as relp,
            tc.tile_pool(name="work", bufs=2) as work,
            tc.tile_pool(name="psum", bufs=2, space="PSUM") as psum,
        ):
            selt = io.tile([CPT, P], bf)
            nc.sync.dma_start(out=selt[:], in_=sel[:])
            scores = io.tile([P, T_PC], f32)

            for g in range(NG):
                htile = spool.tile([P, GROUP * D], bf, tag="h")
                nc.sync.dma_start(
                    out=htile[:].rearrange("p (i d) -> p i d", d=D),
                    in_=h_v[:, g * GROUP : (g + 1) * GROUP, :],
                )
                ttile = spool.tile([P, GROUP * D], bf, tag="t")
                nc.sync.dma_start(
                    out=ttile[:].rearrange("p (i d) -> p i d", d=D),
                    in_=t_v[:, g * GROUP : (g + 1) * GROUP, :],
                )
                rtile = relp.tile([CPT, GROUP * D], bf, tag="r")
                nc.sync.dma_start(out=rtile[:], in_=rel_v[g])

                qtile = work.tile([P, GROUP * D], bf, tag="q")
                nc.vector.tensor_tensor(
                    out=qtile[:], in0=htile[:], in1=ttile[:], op=mult
                )

                for half in range(2):
                    ps = psum.tile([P, 2048], f32, tag="ps")
                    for m4 in range(4):
                        i0 = half * 8 + m4 * 2
                        nc.tensor.matmul(
                            ps[:, m4 * 512 : (m4 + 1) * 512],
                            selt[:],
                            rtile[:, i0 * D : (i0 + 2) * D],
                            start=True,
                            stop=True,
                        )
                    relb = work.tile([P, 2048], bf, tag="b")
                    nc.scalar.activation(
                        out=relb[:], in_=ps[:],
                        func=mybir.ActivationFunctionType.Copy,
                    )
                    stile = work.tile([P, 2048], bf, tag="s")
                    nc.vector.tensor_tensor(
                        out=stile[:],
                        in0=qtile[:, half * 2048 : (half + 1) * 2048],
                        in1=relb[:],
                        op=mult,
                    )
                    c0 = g * GROUP + half * 8
                    nc.vector.tensor_reduce(
                        out=scores[:, c0 : c0 + 8],
                        in_=stile[:].rearrange("p (i d) -> p i d", d=D),
                        axis=mybir.AxisListType.X,
                        op=mybir.AluOpType.add,
                    )

            ubt = io.tile([P, T_PC], f32)
            nc.sync.dma_start(out=ubt[:], in_=ub_v)
            lbt = io.tile([P, T_PC], f32)
            nc.sync.dma_start(out=lbt[:], in_=lb_v)
            c1 = io.tile([P, T_PC], f32)
            nc.vector.tensor_tensor(
                out=c1[:], in0=scores[:], in1=ubt[:], op=mybir.AluOpType.min
            )
            c2 = io.tile([P, T_PC], f32)
            nc.vector.tensor_tensor(
                out=c2[:], in0=c1[:], in1=lbt[:], op=mybir.AluOpType.max
            )
            sig = io.tile([P, T_PC], f32)
            nc.scalar.activation(
                out=sig[:], in_=c2[:], func=mybir.ActivationFunctionType.Sigmoid
            )
            nc.sync.dma_start(out=out_v, in_=sig[:])

    nc.compile()
    return nc


def make_in_maps(inputs: dict):
    """Sort rows by rel id, pack into uniform chunks, build per-core maps.

    Returns (in_maps, order, devrow): sorted row i (original row order[i])
    lands at global device slot devrow[i]; device output is read back with
    out[order] = res_all[devrow].
    """
    import ml_dtypes

    bf16 = ml_dtypes.bfloat16

    head = np.asarray(inputs["head_embeds"], dtype=np.float32).astype(bf16)
    tail = np.asarray(inputs["tail_embeds"], dtype=np.float32).astype(bf16)
    ids = np.asarray(inputs["rel_ids"]).astype(np.int64)
    lower = np.asarray(inputs["lower_bound"], dtype=np.float32)
    upper = np.asarray(inputs["upper_bound"], dtype=np.float32)
    table = np.asarray(inputs["rel_embeds"], dtype=np.float32).astype(bf16)

    order = np.argsort(ids, kind="stable")
    sids = ids[order]
    cnt = np.bincount(sids, minlength=NUM_REL)
    starts = np.zeros(NUM_REL, np.int64)
    starts[1:] = np.cumsum(cnt)[:-1]
    pos_in_rel = np.arange(B, dtype=np.int64) - starts[sids]
    chunks_per_rel = (cnt + CH - 1) // CH
    chunk_base = np.zeros(NUM_REL, np.int64)
    chunk_base[1:] = np.cumsum(chunks_per_rel)[:-1]
    n_chunks = int(chunks_per_rel.sum())
    assert n_chunks <= TOTAL_CHUNKS, n_chunks

    chunk_id = chunk_base[sids] + pos_in_rel // CH
    slot_in_chunk = pos_in_rel % CH

    core = chunk_id // CHUNKS_PER_CORE
    j = chunk_id % CHUNKS_PER_CORE
    t = j // CPT
    k = j % CPT
    p = k * CH + slot_in_chunk
    devrow = core * R + p * T_PC + t  # [B] global device slot per sorted row

    # rel id per chunk (uniform within a chunk; pad chunks use rel 0)
    rel_of_chunk = np.zeros(TOTAL_CHUNKS, np.int64)
    rel_of_chunk[chunk_id] = sids
    cc = np.arange(TOTAL_CHUNKS)
    core_c = cc // CHUNKS_PER_CORE
    j_c = cc % CHUNKS_PER_CORE
    t_c = j_c // CPT
    k_c = j_c % CPT
    relgrid = np.zeros((NCORES, NG, CPT, GROUP), np.int64)
    relgrid[core_c, t_c // GROUP, k_c, t_c % GROUP] = rel_of_chunk
    rel_dev = table[relgrid]  # [NCORES, NG, CPT, GROUP, D] bf16

    h_dev = np.zeros((NCORES * R, D), bf16)
    h_dev[devrow] = head[order]
    t_dev = np.zeros((NCORES * R, D), bf16)
    t_dev[devrow] = tail[order]

    ubv = np.full(B, np.inf, np.float32)
    lbv = np.full(B, -np.inf, np.float32)
    mask = order < N_POS
    ubv[mask] = upper[order[mask]]
    lbv[~mask] = lower[order[~mask] - N_POS]
    ub_dev = np.full(NCORES * R, np.inf, np.float32)
    lb_dev = np.full(NCORES * R, -np.inf, np.float32)
    ub_dev[devrow] = ubv
    lb_dev[devrow] = lbv

    sel = np.zeros((CPT, P), bf16)
    for kk in range(CPT):
        sel[kk, kk * CH : (kk + 1) * CH] = 1.0

    in_maps = []
    for c in range(NCORES):
        lo = c * R
        hi = lo + R
        in_maps.append(
            {
                "h": np.ascontiguousarray(h_dev[lo:hi]),
                "t": np.ascontiguousarray(t_dev[lo:hi]),
                "rel": np.ascontiguousarray(
                    rel_dev[c].reshape(NG, CPT, GROUP * D)
                ),
                "sel": sel,
                "ub": np.ascontiguousarray(ub_dev[lo:hi]),
                "lb": np.ascontiguousarray(lb_dev[lo:hi]),
            }
        )
    return in_maps, order, devrow


def _run(inputs: dict, trace: bool = False, tmpdir: str | None = None):
    nc = build_program()
    in_maps, order, devrow = make_in_maps(inputs)
    res = run_bass_kernel_spmd(
        nc, in_maps, list(range(NCORES)), trace=trace, tmpdir=tmpdir
    )
    res_all = np.concatenate(
        [np.asarray(res.results[c]["out"]) for c in range(NCORES)]
    )
    out = np.empty(B, np.float32)
    out[order] = res_all[devrow]
    return out, res


def kernel(**inputs) -> np.ndarray:
    out, _ = _run(inputs)
    return out
